# revision 29
# baseline (speedup 1.0000x reference)
"""Trainium2 Bass kernel for a 4-layer NeRF-style MLP.

    y = relu(relu(relu(x@W1.T+b1)@W2.T+b2)@W3.T+b3)@W4.T+b4
    x: [1048576, 6] fp32 -> y: [1048576, 4] fp32

Strategy: pure data parallel over 8 NeuronCores (131072 rows each).
On-device layout keeps features on SBUF partitions and rows on the free
dim, so every layer's PSUM output is directly the next layer's matmul
rhs -- no transposes anywhere.

Per core, rows are processed in groups of 4 chunks x 512 rows, split
into two independent half-group chains (a: chunks 0-1, b: chunks 2-3),
each owning a 2-bank PSUM tile.  With pool bufs=2, four half-chains are
in flight, which hides each PSUM->SBUF eviction's latency behind the
other chains' matmuls (the evictions, on ScalarE/VectorE at ~1 col/cyc
from PSUM, are the throughput wall of this dataflow: 3 layers x 2048
cols per group across two engines ~= 3.4 us/group).

  - layer 1 (K=6+1): the 4 chunks are packed into the four 32-row PE
    groups (tile_position row packing) and run concurrently; the bias is
    folded into the matmul via a constant ones-row in x (K=7).
  - layers 2/3 (K=128): one matmul per chunk, float32r (1 cycle/row).
  - layer 4 is computed transposed (h3-slice stationary, W4.T moving,
    N=4) with h3/W4 in bf16 so the 16 LDWEIGHTS per group get Fast
    Weight Load (2 cols/cycle); the group's output is a dense [128, 32]
    PSUM block per half, so its eviction is nearly free.  b4 is added on
    the host.
  - evictions are fused bias+ReLU ops; the a/b chain <-> ScalarE/VectorE
    assignment alternates by group parity.
  - all weights/biases ship as one packed [128, 388] input (W4's bf16
    pairs bitcast into two f32 columns) to minimize per-dispatch PJRT
    operand overhead.
"""

import numpy as np

N = 1048576
CORES = 8
R = N // CORES            # rows per core
CHUNK = 512               # rows per matmul (one PSUM bank of fp32)
GPC = 4                   # chunks per group
GROUPS = R // (CHUNK * GPC)   # 64
GW = GPC * CHUNK          # 2048 columns per group
REPEAT = 1                # times to run the whole compute body (bench only)

_CACHE = {}


def _build():
    import concourse.bacc as bacc
    import concourse.mybir as mybir
    import concourse.tile as tile

    f32 = mybir.dt.float32
    f32r = mybir.dt.float32r
    bf16 = mybir.dt.bfloat16
    Relu = mybir.ActivationFunctionType.Relu
    op_add = mybir.AluOpType.add
    op_max = mybir.AluOpType.max

    nc = bacc.Bacc("TRN2", target_bir_lowering=False, debug=False)

    xin = nc.dram_tensor(
        "xin", [GROUPS // 4, GPC, 7, 4 * CHUNK], f32r, kind="ExternalInput"
    ).ap()
    wpack = nc.dram_tensor(
        "wpack", [128, 390], f32r, kind="ExternalInput"
    ).ap()  # w1 | w2 | w3 | b2 | b3 | w4
    yout = nc.dram_tensor(
        "yout", [GROUPS // 4, 2, 128, 128], bf16, kind="ExternalOutput"
    ).ap()

    with tile.TileContext(nc) as tc:
        with (
            tc.tile_pool(name="const", bufs=1) as cpool,
            tc.tile_pool(name="x", bufs=4) as xpool,
            tc.tile_pool(name="h", bufs=6) as hpool,
            tc.tile_pool(name="o", bufs=4) as opool,
            tc.tile_pool(name="psum", bufs=2, space="PSUM") as ppool,
        ):
            wps = cpool.tile([128, 390], f32r, tag="wp")
            nc.sync.dma_start(out=wps[:], in_=wpack)
            # w4 ships as f32 inside wpack; cast to bf16 on device (SWDGE
            # cast DMA) so layer 4's LDWEIGHTS get Fast Weight Load
            w4t_ = cpool.tile([128, 4], bf16, tag="w4")
            nc.gpsimd.dma_start(
                out=w4t_[:, :], in_=wpack[:, 386:390].bitcast(f32)
            )
            w4s = w4t_[:, :]
            w1s = wps[:, 0:128]
            w2s = wps[:, 128:256]
            w3s = wps[:, 256:384]
            b2s = wps[:, 384:385].bitcast(f32)
            b3s = wps[:, 385:386].bitcast(f32)

            w1r = w1s.rearrange("(a b) c -> a b c", b=32)

            HW = GW // 2  # 1024 columns: each half-group (2 chunks)
            st = {}       # per-group in-flight tiles
            xts = {}      # x tile per 4-group block
            oab = {}      # output accumulation tiles per 4-group block

            def evict_relu(use_act, out_ap, in_ap, bias_ap):
                """bias+ReLU PSUM->SBUF eviction on either engine."""
                if use_act:
                    if bias_ap is None:
                        nc.scalar.activation(out_ap, in_ap, Relu)
                    else:
                        nc.scalar.activation(out_ap, in_ap, Relu, bias=bias_ap)
                elif bias_ap is None:
                    nc.vector.tensor_scalar(
                        out=out_ap,
                        in0=in_ap,
                        scalar1=0.0,
                        scalar2=None,
                        op0=op_max,
                    )
                else:
                    nc.vector.tensor_scalar(
                        out=out_ap,
                        in0=in_ap,
                        scalar1=bias_ap,
                        scalar2=0.0,
                        op0=op_add,
                        op1=op_max,
                    )

            def evict_copy(use_act, out_ap, in_ap):
                if use_act:
                    nc.scalar.activation(
                        out_ap, in_ap, mybir.ActivationFunctionType.Copy
                    )
                else:
                    nc.vector.tensor_copy(out=out_ap, in_=in_ap)

            def load_x(blk):
                """DMA one 4-group block of x into SBUF."""
                if blk >= GROUPS // 4 or blk in xts:
                    return
                xt = xpool.tile([128, 4 * CHUNK], f32r, tag="x")
                xtr = xt.rearrange("(a b) c -> a b c", b=32)
                for c in range(GPC):
                    nc.sync.dma_start(out=xtr[c, 0:7, :], in_=xin[blk, c])
                xts[blk] = xtr

            def front_a(g):
                """layer-1 matmuls + L1 eviction (x prefetched a block
                ahead so L1 never waits on the DMA)."""
                q = g % 4
                if q == 0:
                    load_x(g // 4)      # no-op except for block 0
                    load_x(g // 4 + 1)  # prefetch next block
                xtr = xts[g // 4]
                # two independent half-group chains: a (chunks 0-1, all
                # evictions on ScalarE) and b (chunks 2-3, VectorE).  Each
                # half owns a 2-bank PSUM tile; with bufs=2 four half-
                # chains are in flight, hiding each eviction's latency.
                pa = ppool.tile([128, HW], f32, tag="pa")
                pb = ppool.tile([128, HW], f32, tag="pb")
                for c in range(GPC):
                    dst = pa if c < 2 else pb
                    off = (c % 2) * CHUNK
                    nc.tensor.matmul(
                        dst[:, off : off + CHUNK],
                        lhsT=w1r[c, 0:7, :],
                        rhs=xtr[c, 0:7, q * CHUNK : (q + 1) * CHUNK],
                        start=True,
                        stop=True,
                        tile_position=(32 * c, 0),
                    )
                ha = hpool.tile([128, HW], f32r, tag="ha")
                hb = hpool.tile([128, HW], f32r, tag="hb")
                act_a = g % 2 == 0  # alternate engine<->chain to balance
                evict_relu(act_a, ha[:, :], pa[:, :], None)
                evict_relu(not act_a, hb[:, :], pb[:, :], None)
                st[g] = {"pa": pa, "pb": pb, "ha": ha, "hb": hb}

            def front_b(g):
                """layer-2 matmuls + L2 eviction."""
                s = st[g]
                for c in range(GPC):
                    dst = s["pa"] if c < 2 else s["pb"]
                    src_h = s["ha"] if c < 2 else s["hb"]
                    off = (c % 2) * CHUNK
                    nc.tensor.matmul(
                        dst[:, off : off + CHUNK],
                        lhsT=w2s,
                        rhs=src_h[:, off : off + CHUNK],
                        start=True,
                        stop=True,
                    )
                h2a = hpool.tile([128, HW], f32r, tag="h2a")
                h2b = hpool.tile([128, HW], f32r, tag="h2b")
                act_a = g % 2 == 0
                evict_relu(act_a, h2a[:, :], s["pa"][:, :], b2s)
                evict_relu(not act_a, h2b[:, :], s["pb"][:, :], b2s)
                s["h2a"], s["h2b"] = h2a, h2b

            def back_a(g):
                """layer-3 matmuls + L3 eviction (bf16 h3 -> FWL in L4)."""
                s = st[g]
                for c in range(GPC):
                    dst = s["pa"] if c < 2 else s["pb"]
                    src_h = s["h2a"] if c < 2 else s["h2b"]
                    off = (c % 2) * CHUNK
                    nc.tensor.matmul(
                        dst[:, off : off + CHUNK],
                        lhsT=w3s,
                        rhs=src_h[:, off : off + CHUNK],
                        start=True,
                        stop=True,
                    )
                h3a = hpool.tile([128, HW], bf16, tag="h3a")
                h3b = hpool.tile([128, HW], bf16, tag="h3b")
                act_a = g % 2 == 0
                evict_relu(act_a, h3a[:, :], s["pa"][:, :], b3s)
                evict_relu(not act_a, h3b[:, :], s["pb"][:, :], b3s)
                s["h3a"], s["h3b"] = h3a, h3b

            def back_b(g):
                """layer 4 (transposed, bf16 FWL), output copy + DMA."""
                s = st.pop(g)
                q = g % 4
                for sl in range(16):
                    dst = s["pa"] if sl < 8 else s["pb"]
                    src_h = s["h3a"] if sl < 8 else s["h3b"]
                    off = 128 * (sl % 8)
                    nc.tensor.matmul(
                        dst[:, 4 * (sl % 8) : 4 * (sl % 8) + 4],
                        lhsT=src_h[:, off : off + 128],
                        rhs=w4s[:, :],
                        start=True,
                        stop=True,
                        skip_group_check=True,
                    )
                if q == 0:
                    ota = opool.tile([128, 128], bf16, tag="oa")
                    otb = opool.tile([128, 128], bf16, tag="ob")
                    oab[g // 4] = (ota, otb)
                ota, otb = oab[g // 4]
                act_a = g % 2 == 0
                evict_copy(act_a, ota[:, 32 * q : 32 * q + 32], s["pa"][:, 0:32])
                evict_copy(not act_a, otb[:, 32 * q : 32 * q + 32], s["pb"][:, 0:32])
                if q == 3:
                    nc.sync.dma_start(out=yout[g // 4, 0], in_=ota[:])
                    nc.sync.dma_start(out=yout[g // 4, 1], in_=otb[:])
                    del oab[g // 4], xts[g // 4]

            # two-stage software pipeline over groups (the scheduler also
            # reorders by readiness; the pa/pb half-chains are what give
            # it slack to fill eviction waits)
            for gg in [g for _ in range(REPEAT) for g in range(GROUPS + 1)]:
                if gg < GROUPS:
                    front_a(gg)
                if gg >= 1:
                    back_a(gg - 1)
                if gg < GROUPS:
                    front_b(gg)
                if gg >= 1:
                    back_b(gg - 1)

    nc.compile()
    return nc


def _prep_in_maps(x, W1, b1, W2, b2, W3, b3, W4, b4):
    x = np.ascontiguousarray(np.asarray(x, dtype=np.float32))

    wp = np.zeros((128, 390), np.float32)
    W1T = np.asarray(W1, np.float32).T  # [6, 128]
    for g in range(GPC):
        wp[32 * g : 32 * g + 6, 0:128] = W1T
        wp[32 * g + 6, 0:128] = np.asarray(b1, np.float32)
    wp[:, 128:256] = np.asarray(W2, np.float32).T
    wp[:, 256:384] = np.asarray(W3, np.float32).T
    wp[:, 384] = np.asarray(b2, np.float32)
    wp[:, 385] = np.asarray(b3, np.float32)
    wp[:, 386:390] = np.asarray(W4, np.float32).T

    in_maps = []
    for c in range(CORES):
        xc = x[c * R : (c + 1) * R]  # [R, 6]
        # xin[xg, g, k, q*CHUNK + j] = xc[((xg*4 + q)*GPC + g)*CHUNK + j, k]
        xr = xc.reshape(GROUPS // 4, 4, GPC, CHUNK, 6).transpose(0, 2, 4, 1, 3)
        xr = xr.reshape(GROUPS // 4, GPC, 6, 4 * CHUNK)
        xi = np.empty((GROUPS // 4, GPC, 7, 4 * CHUNK), np.float32)
        xi[:, :, 0:6, :] = xr
        xi[:, :, 6, :] = 1.0
        in_maps.append({"xin": xi, "wpack": wp})
    return in_maps


def _execute(in_maps, trace=False):
    from concourse.bass_utils import run_bass_kernel_spmd

    if "nc" not in _CACHE:
        _CACHE["nc"] = _build()
    return run_bass_kernel_spmd(
        _CACHE["nc"], in_maps, list(range(CORES)), trace=trace
    )


def bench(in_maps, iters=20):
    """Measure the per-iteration device-side execution time of the kernel.

    The NeuronCores are reached through an axon tunnel whose host<->device
    round-trip latency is ~60 ms — three orders of magnitude above the
    kernel itself — so timing one synchronous dispatch measures the
    network, not the hardware.  Instead we enqueue N dispatches
    back-to-back (device-resident inputs, one final block_until_ready) so
    consecutive NEFF executions pipeline on-device, and recover the
    marginal per-iteration cost as the slope between a short and a long
    pipelined batch: slope = (T(N2) - T(N1)) / (N2 - N1).  The one-time
    tunnel round trip cancels in the difference.  Batches are repeated
    interleaved and min-aggregated to reject one-sided scheduling noise.

    Returns [slope_seconds] (list, for min() compatibility).
    """
    import time

    import jax
    from jax.experimental.shard_map import shard_map
    from jax.sharding import Mesh, NamedSharding, PartitionSpec

    import concourse.mybir as mybir
    from concourse import bass2jax

    if "nc" not in _CACHE:
        _CACHE["nc"] = _build()
    nc = _CACHE["nc"]
    bass2jax.install_neuronx_cc_hook()

    in_names, out_names, out_avals = [], [], []
    for alloc in nc.m.functions[0].allocations:
        if not isinstance(alloc, mybir.MemoryLocationSet):
            continue
        name = alloc.memorylocations[0].name
        pid = nc.partition_id_tensor.name if nc.partition_id_tensor else None
        if alloc.kind == "ExternalInput":
            if name != pid:
                in_names.append(name)
        elif alloc.kind == "ExternalOutput":
            out_names.append(name)
            out_avals.append(
                jax.core.ShapedArray(
                    tuple(alloc.tensor_shape), mybir.dt.np(alloc.dtype)
                )
            )
    n_params = len(in_names)
    all_names = tuple(in_names + out_names)

    def _body(*args):
        operands = list(args)
        if nc.partition_id_tensor is not None:
            operands.append(bass2jax.partition_id_tensor())
        outs = bass2jax._bass_exec_p.bind(
            *operands,
            out_avals=tuple(out_avals),
            in_names=all_names
            + ((nc.partition_id_tensor.name,) if nc.partition_id_tensor else ()),
            out_names=tuple(out_names),
            lowering_input_output_aliases=(),
            sim_require_finite=True,
            sim_require_nnan=True,
            nc=nc,
        )
        return tuple(outs)

    devices = jax.devices()[:CORES]
    mesh = Mesh(np.asarray(devices), ("core",))
    in_specs = (PartitionSpec("core"),) * (n_params + len(out_names))
    out_specs = (PartitionSpec("core"),) * len(out_names)
    sm = shard_map(
        _body, mesh=mesh, in_specs=in_specs, out_specs=out_specs, check_rep=False
    )

    concat_in = [
        np.concatenate([np.asarray(in_maps[c][n]) for c in range(CORES)], axis=0)
        for n in in_names
    ]
    zeros = [
        np.zeros((CORES * av.shape[0], *av.shape[1:]), av.dtype) for av in out_avals
    ]
    sh = NamedSharding(mesh, PartitionSpec("core"))
    dev_in = [jax.device_put(a, sh) for a in concat_in]
    dev_zeros = [jax.device_put(z, sh) for z in zeros]

    fn = bass2jax.fast_dispatch_compile(
        lambda: jax.jit(sm, keep_unused=True).lower(*dev_in, *dev_zeros).compile()
    )

    def batch(n):
        t0 = time.perf_counter()
        out = None
        for _ in range(n):
            out = fn(*dev_in, *dev_zeros)
        jax.block_until_ready(out)
        return time.perf_counter() - t0

    batch(2)  # warmup
    n1, n2, reps = 10, 100, max(8, iters // 3)
    t1s, t2s = [], []
    for _ in range(reps):
        t1s.append(batch(n1))
        t2s.append(batch(n2))
    slope = (min(t2s) - min(t1s)) / (n2 - n1)
    slope = max(slope, 1e-9)
    print(
        f"bench: T({n1}) {[round(t * 1e3, 2) for t in t1s]} ms, "
        f"T({n2}) {[round(t * 1e3, 2) for t in t2s]} ms"
    )
    return [slope]


def kernel(**inputs):
    in_maps = _prep_in_maps(
        inputs["x"],
        inputs["W1"],
        inputs["b1"],
        inputs["W2"],
        inputs["b2"],
        inputs["W3"],
        inputs["b3"],
        inputs["W4"],
        inputs["b4"],
    )
    results = _execute(in_maps).results
    outs = []
    for c in range(CORES):
        # yout dims: (xg, half, p, (q, s4, k)); group = xg*4 + q,
        # row = group*2048 + half*1024 + s4*128 + p
        yo = (
            np.asarray(results[c]["yout"])
            .astype(np.float32)
            .reshape(GROUPS // 4, 2, 128, 4, 8, 4)
        )
        outs.append(yo.transpose(0, 3, 1, 4, 2, 5).reshape(R, 4))
    y = np.concatenate(outs, axis=0)
    y += np.asarray(inputs["b4"], np.float32)  # layer-4 bias, added on host
    return np.ascontiguousarray(y.astype(np.float32))



# revision 31
# speedup vs baseline: 1.0486x; 1.0486x over previous
"""Trainium2 Bass kernel for a 4-layer NeRF-style MLP.

    y = relu(relu(relu(x@W1.T+b1)@W2.T+b2)@W3.T+b3)@W4.T+b4
    x: [1048576, 6] fp32 -> y: [1048576, 4] fp32

Strategy: pure data parallel over 8 NeuronCores (131072 rows each).
On-device layout keeps features on SBUF partitions and rows on the free
dim, so every layer's PSUM output is directly the next layer's matmul
rhs -- no transposes anywhere.

Per core, rows are processed in groups of 4 chunks x 512 rows, split
into two independent half-group chains (a: chunks 0-1, b: chunks 2-3),
each owning a 2-bank PSUM tile.  With pool bufs=2, four half-chains are
in flight, which hides each PSUM->SBUF eviction's latency behind the
other chains' matmuls (the evictions, on ScalarE/VectorE at ~1 col/cyc
from PSUM, are the throughput wall of this dataflow: 3 layers x 2048
cols per group across two engines ~= 3.4 us/group).

  - layer 1 (K=6+1): the 4 chunks are packed into the four 32-row PE
    groups (tile_position row packing) and run concurrently; the bias is
    folded into the matmul via a constant ones-row in x (K=7).
  - layers 2/3 (K=128): one matmul per chunk, float32r (1 cycle/row).
  - layer 4 is computed transposed (h3-slice stationary, W4.T moving,
    N=4) with h3/W4 in bf16 so the 16 LDWEIGHTS per group get Fast
    Weight Load (2 cols/cycle); the group's output is a dense [128, 32]
    PSUM block per half, so its eviction is nearly free.  b4 is added on
    the host.
  - evictions are fused bias+ReLU ops; the a/b chain <-> ScalarE/VectorE
    assignment alternates by group parity.
  - all weights/biases ship as one packed [128, 388] input (W4's bf16
    pairs bitcast into two f32 columns) to minimize per-dispatch PJRT
    operand overhead.
"""

import numpy as np

N = 1048576
CORES = 8
R = N // CORES            # rows per core
CHUNK = 512               # rows per matmul (one PSUM bank of fp32)
GPC = 4                   # chunks per group
GROUPS = R // (CHUNK * GPC)   # 64
GW = GPC * CHUNK          # 2048 columns per group
REPEAT = 1                # times to run the whole compute body (bench only)

_CACHE = {}


def _build():
    import concourse.bacc as bacc
    import concourse.mybir as mybir
    import concourse.tile as tile

    f32 = mybir.dt.float32
    f32r = mybir.dt.float32r
    bf16 = mybir.dt.bfloat16
    Relu = mybir.ActivationFunctionType.Relu
    op_add = mybir.AluOpType.add
    op_max = mybir.AluOpType.max

    nc = bacc.Bacc("TRN2", target_bir_lowering=False, debug=False)

    xin = nc.dram_tensor(
        "xin", [GROUPS // 4, GPC, 7, 4 * CHUNK], f32r, kind="ExternalInput"
    ).ap()
    wpack = nc.dram_tensor(
        "wpack", [128, 390], f32r, kind="ExternalInput"
    ).ap()  # w1 | w2 | w3 | b2 | b3 | w4
    yout = nc.dram_tensor(
        "yout", [GROUPS // 4, 2, 128, 128], bf16, kind="ExternalOutput"
    ).ap()

    with tile.TileContext(nc) as tc:
        with (
            tc.tile_pool(name="const", bufs=1) as cpool,
            tc.tile_pool(name="x", bufs=4) as xpool,
            tc.tile_pool(name="h", bufs=6) as hpool,
            tc.tile_pool(name="o", bufs=4) as opool,
            tc.tile_pool(name="psum", bufs=2, space="PSUM") as ppool,
        ):
            wps = cpool.tile([128, 390], f32r, tag="wp")
            nc.sync.dma_start(out=wps[:], in_=wpack)
            # w4 ships as f32 inside wpack; cast to bf16 on device (SWDGE
            # cast DMA) so layer 4's LDWEIGHTS get Fast Weight Load
            w4t_ = cpool.tile([128, 4], bf16, tag="w4")
            nc.gpsimd.dma_start(
                out=w4t_[:, :], in_=wpack[:, 386:390].bitcast(f32)
            )
            w4s = w4t_[:, :]
            w1s = wps[:, 0:128]
            w2s = wps[:, 128:256]
            w3s = wps[:, 256:384]
            b2s = wps[:, 384:385].bitcast(f32)
            b3s = wps[:, 385:386].bitcast(f32)

            w1r = w1s.rearrange("(a b) c -> a b c", b=32)

            HW = GW // 2  # 1024 columns: each half-group (2 chunks)
            st = {}       # per-group in-flight tiles
            xts = {}      # x tile per 4-group block
            oab = {}      # output accumulation tiles per 4-group block

            def evict_relu(use_act, out_ap, in_ap, bias_ap):
                """bias+ReLU PSUM->SBUF eviction on either engine."""
                if use_act:
                    if bias_ap is None:
                        nc.scalar.activation(out_ap, in_ap, Relu)
                    else:
                        nc.scalar.activation(out_ap, in_ap, Relu, bias=bias_ap)
                elif bias_ap is None:
                    nc.vector.tensor_scalar(
                        out=out_ap,
                        in0=in_ap,
                        scalar1=0.0,
                        scalar2=None,
                        op0=op_max,
                    )
                else:
                    nc.vector.tensor_scalar(
                        out=out_ap,
                        in0=in_ap,
                        scalar1=bias_ap,
                        scalar2=0.0,
                        op0=op_add,
                        op1=op_max,
                    )

            def evict_copy(use_act, out_ap, in_ap):
                if use_act:
                    nc.scalar.activation(
                        out_ap, in_ap, mybir.ActivationFunctionType.Copy
                    )
                else:
                    nc.vector.tensor_copy(out=out_ap, in_=in_ap)

            def load_x(blk):
                """DMA one 4-group block of x into SBUF."""
                if blk >= GROUPS // 4 or blk in xts:
                    return
                xt = xpool.tile([128, 4 * CHUNK], f32r, tag="x")
                xtr = xt.rearrange("(a b) c -> a b c", b=32)
                for c in range(GPC):
                    nc.sync.dma_start(out=xtr[c, 0:7, :], in_=xin[blk, c])
                xts[blk] = xtr

            def front_a(g):
                """layer-1 matmuls + L1 eviction (x prefetched a block
                ahead so L1 never waits on the DMA)."""
                q = g % 4
                if q == 0:
                    load_x(g // 4)      # no-op except for block 0
                    load_x(g // 4 + 1)  # prefetch next block
                xtr = xts[g // 4]
                # two independent half-group chains: a (chunks 0-1, all
                # evictions on ScalarE) and b (chunks 2-3, VectorE).  Each
                # half owns a 2-bank PSUM tile; with bufs=2 four half-
                # chains are in flight, hiding each eviction's latency.
                pa = ppool.tile([128, HW], f32, tag="pa")
                pb = ppool.tile([128, HW], f32, tag="pb")
                for c in range(GPC):
                    dst = pa if c < 2 else pb
                    off = (c % 2) * CHUNK
                    nc.tensor.matmul(
                        dst[:, off : off + CHUNK],
                        lhsT=w1r[c, 0:7, :],
                        rhs=xtr[c, 0:7, q * CHUNK : (q + 1) * CHUNK],
                        start=True,
                        stop=True,
                        tile_position=(32 * c, 0),
                    )
                ha = hpool.tile([128, HW], f32r, tag="ha")
                hb = hpool.tile([128, HW], f32r, tag="hb")
                act_a = g % 2 == 0  # alternate engine<->chain to balance
                evict_relu(act_a, ha[:, :], pa[:, :], None)
                evict_relu(not act_a, hb[:, :], pb[:, :], None)
                st[g] = {"pa": pa, "pb": pb, "ha": ha, "hb": hb}

            def front_b(g):
                """layer-2 matmuls + L2 eviction."""
                s = st[g]
                for c in range(GPC):
                    dst = s["pa"] if c < 2 else s["pb"]
                    src_h = s["ha"] if c < 2 else s["hb"]
                    off = (c % 2) * CHUNK
                    nc.tensor.matmul(
                        dst[:, off : off + CHUNK],
                        lhsT=w2s,
                        rhs=src_h[:, off : off + CHUNK],
                        start=True,
                        stop=True,
                    )
                h2a = hpool.tile([128, HW], f32r, tag="h2a")
                h2b = hpool.tile([128, HW], f32r, tag="h2b")
                act_a = g % 2 == 0
                evict_relu(act_a, h2a[:, :], s["pa"][:, :], b2s)
                evict_relu(not act_a, h2b[:, :], s["pb"][:, :], b2s)
                s["h2a"], s["h2b"] = h2a, h2b

            def back_a(g):
                """layer-3 matmuls + L3 eviction (bf16 h3 -> FWL in L4)."""
                s = st[g]
                for c in range(GPC):
                    dst = s["pa"] if c < 2 else s["pb"]
                    src_h = s["h2a"] if c < 2 else s["h2b"]
                    off = (c % 2) * CHUNK
                    nc.tensor.matmul(
                        dst[:, off : off + CHUNK],
                        lhsT=w3s,
                        rhs=src_h[:, off : off + CHUNK],
                        start=True,
                        stop=True,
                    )
                h3a = hpool.tile([128, HW], bf16, tag="h3a")
                h3b = hpool.tile([128, HW], bf16, tag="h3b")
                act_a = g % 2 == 0
                evict_relu(act_a, h3a[:, :], s["pa"][:, :], b3s)
                evict_relu(not act_a, h3b[:, :], s["pb"][:, :], b3s)
                s["h3a"], s["h3b"] = h3a, h3b

            def back_b(g):
                """layer 4 (transposed, bf16 FWL), output copy + DMA."""
                s = st.pop(g)
                q = g % 4
                for sl in range(16):
                    dst = s["pa"] if sl < 8 else s["pb"]
                    src_h = s["h3a"] if sl < 8 else s["h3b"]
                    off = 128 * (sl % 8)
                    nc.tensor.matmul(
                        dst[:, 4 * (sl % 8) : 4 * (sl % 8) + 4],
                        lhsT=src_h[:, off : off + 128],
                        rhs=w4s[:, :],
                        start=True,
                        stop=True,
                        skip_group_check=True,
                    )
                if q == 0:
                    ota = opool.tile([128, 128], bf16, tag="oa")
                    otb = opool.tile([128, 128], bf16, tag="ob")
                    oab[g // 4] = (ota, otb)
                ota, otb = oab[g // 4]
                act_a = g % 2 == 0
                evict_copy(act_a, ota[:, 32 * q : 32 * q + 32], s["pa"][:, 0:32])
                evict_copy(not act_a, otb[:, 32 * q : 32 * q + 32], s["pb"][:, 0:32])
                if q == 3:
                    nc.sync.dma_start(out=yout[g // 4, 0], in_=ota[:])
                    nc.sync.dma_start(out=yout[g // 4, 1], in_=otb[:])
                    del oab[g // 4], xts[g // 4]

            # two-stage software pipeline over groups (the scheduler also
            # reorders by readiness; the pa/pb half-chains are what give
            # it slack to fill eviction waits)
            for gg in [g for _ in range(REPEAT) for g in range(GROUPS + 1)]:
                if gg < GROUPS:
                    front_a(gg)
                if gg >= 1:
                    back_a(gg - 1)
                if gg < GROUPS:
                    front_b(gg)
                if gg >= 1:
                    back_b(gg - 1)

    nc.compile()
    return nc


def _prep_in_maps(x, W1, b1, W2, b2, W3, b3, W4, b4):
    x = np.ascontiguousarray(np.asarray(x, dtype=np.float32))

    wp = np.zeros((128, 390), np.float32)
    W1T = np.asarray(W1, np.float32).T  # [6, 128]
    for g in range(GPC):
        wp[32 * g : 32 * g + 6, 0:128] = W1T
        wp[32 * g + 6, 0:128] = np.asarray(b1, np.float32)
    wp[:, 128:256] = np.asarray(W2, np.float32).T
    wp[:, 256:384] = np.asarray(W3, np.float32).T
    wp[:, 384] = np.asarray(b2, np.float32)
    wp[:, 385] = np.asarray(b3, np.float32)
    wp[:, 386:390] = np.asarray(W4, np.float32).T

    in_maps = []
    for c in range(CORES):
        xc = x[c * R : (c + 1) * R]  # [R, 6]
        # xin[xg, g, k, q*CHUNK + j] = xc[((xg*4 + q)*GPC + g)*CHUNK + j, k]
        xr = xc.reshape(GROUPS // 4, 4, GPC, CHUNK, 6).transpose(0, 2, 4, 1, 3)
        xr = xr.reshape(GROUPS // 4, GPC, 6, 4 * CHUNK)
        xi = np.empty((GROUPS // 4, GPC, 7, 4 * CHUNK), np.float32)
        xi[:, :, 0:6, :] = xr
        xi[:, :, 6, :] = 1.0
        in_maps.append({"xin": xi, "wpack": wp})
    return in_maps


def _execute(in_maps, trace=False):
    from concourse.bass_utils import run_bass_kernel_spmd

    if "nc" not in _CACHE:
        _CACHE["nc"] = _build()
    return run_bass_kernel_spmd(
        _CACHE["nc"], in_maps, list(range(CORES)), trace=trace
    )


def bench(in_maps, iters=20):
    """Measure the per-iteration device-side execution time of the kernel.

    The NeuronCores are reached through an axon tunnel whose host<->device
    round-trip latency is ~60 ms — three orders of magnitude above the
    kernel itself — so timing one synchronous dispatch measures the
    network, not the hardware.  Instead we enqueue N dispatches
    back-to-back (device-resident inputs, one final block_until_ready) so
    consecutive NEFF executions pipeline on-device, and recover the
    marginal per-iteration cost as the slope between a short and a long
    pipelined batch: slope = (T(N2) - T(N1)) / (N2 - N1).  The one-time
    tunnel round trip cancels in the difference.  Batches are repeated
    interleaved and min-aggregated to reject one-sided scheduling noise.

    Returns [slope_seconds] (list, for min() compatibility).
    """
    import time

    import jax
    from jax.experimental.shard_map import shard_map
    from jax.sharding import Mesh, NamedSharding, PartitionSpec

    import concourse.mybir as mybir
    from concourse import bass2jax

    if "nc" not in _CACHE:
        _CACHE["nc"] = _build()
    nc = _CACHE["nc"]
    bass2jax.install_neuronx_cc_hook()

    in_names, out_names, out_avals = [], [], []
    for alloc in nc.m.functions[0].allocations:
        if not isinstance(alloc, mybir.MemoryLocationSet):
            continue
        name = alloc.memorylocations[0].name
        pid = nc.partition_id_tensor.name if nc.partition_id_tensor else None
        if alloc.kind == "ExternalInput":
            if name != pid:
                in_names.append(name)
        elif alloc.kind == "ExternalOutput":
            out_names.append(name)
            out_avals.append(
                jax.core.ShapedArray(
                    tuple(alloc.tensor_shape), mybir.dt.np(alloc.dtype)
                )
            )
    n_params = len(in_names)
    all_names = tuple(in_names + out_names)

    def _body(*args):
        operands = list(args)
        if nc.partition_id_tensor is not None:
            operands.append(bass2jax.partition_id_tensor())
        outs = bass2jax._bass_exec_p.bind(
            *operands,
            out_avals=tuple(out_avals),
            in_names=all_names
            + ((nc.partition_id_tensor.name,) if nc.partition_id_tensor else ()),
            out_names=tuple(out_names),
            lowering_input_output_aliases=(),
            sim_require_finite=True,
            sim_require_nnan=True,
            nc=nc,
        )
        return tuple(outs)

    devices = jax.devices()[:CORES]
    mesh = Mesh(np.asarray(devices), ("core",))
    in_specs = (PartitionSpec("core"),) * (n_params + len(out_names))
    out_specs = (PartitionSpec("core"),) * len(out_names)
    sm = shard_map(
        _body, mesh=mesh, in_specs=in_specs, out_specs=out_specs, check_rep=False
    )

    concat_in = [
        np.concatenate([np.asarray(in_maps[c][n]) for c in range(CORES)], axis=0)
        for n in in_names
    ]
    zeros = [
        np.zeros((CORES * av.shape[0], *av.shape[1:]), av.dtype) for av in out_avals
    ]
    sh = NamedSharding(mesh, PartitionSpec("core"))
    dev_in = [jax.device_put(a, sh) for a in concat_in]
    dev_zeros = [jax.device_put(z, sh) for z in zeros]

    fn = bass2jax.fast_dispatch_compile(
        lambda: jax.jit(sm, keep_unused=True).lower(*dev_in, *dev_zeros).compile()
    )

    def batch(n):
        t0 = time.perf_counter()
        out = None
        for _ in range(n):
            out = fn(*dev_in, *dev_zeros)
        jax.block_until_ready(out)
        return time.perf_counter() - t0

    batch(2)  # warmup
    n1, n2, reps = 10, 100, max(8, iters // 3)
    t1s, t2s = [], []
    for _ in range(reps):
        t1s.append(batch(n1))
        t2s.append(batch(n2))
    slope = (min(t2s) - min(t1s)) / (n2 - n1)
    slope = max(slope, 1e-9)
    print(
        f"bench: T({n1}) {[round(t * 1e3, 2) for t in t1s]} ms, "
        f"T({n2}) {[round(t * 1e3, 2) for t in t2s]} ms"
    )
    return [slope]


def kernel(**inputs):
    in_maps = _prep_in_maps(
        inputs["x"],
        inputs["W1"],
        inputs["b1"],
        inputs["W2"],
        inputs["b2"],
        inputs["W3"],
        inputs["b3"],
        inputs["W4"],
        inputs["b4"],
    )
    results = _execute(in_maps).results
    outs = []
    for c in range(CORES):
        # yout dims: (xg, half, p, (q, s4, k)); group = xg*4 + q,
        # row = group*2048 + half*1024 + s4*128 + p
        yo = (
            np.asarray(results[c]["yout"])
            .astype(np.float32)
            .reshape(GROUPS // 4, 2, 128, 4, 8, 4)
        )
        outs.append(yo.transpose(0, 3, 1, 4, 2, 5).reshape(R, 4))
    y = np.concatenate(outs, axis=0)
    y += np.asarray(inputs["b4"], np.float32)  # layer-4 bias, added on host
    return np.ascontiguousarray(y.astype(np.float32))



# revision 33
# speedup vs baseline: 1.0648x; 1.0155x over previous
"""Trainium2 Bass kernel for a 4-layer NeRF-style MLP.

    y = relu(relu(relu(x@W1.T+b1)@W2.T+b2)@W3.T+b3)@W4.T+b4
    x: [1048576, 6] fp32 -> y: [1048576, 4] fp32

Strategy: pure data parallel over 8 NeuronCores (131072 rows each).
On-device layout keeps features on SBUF partitions and rows on the free
dim, so every layer's PSUM output is directly the next layer's matmul
rhs -- no transposes anywhere.

Per core, rows are processed in groups of 4 chunks x 512 rows, split
into two independent half-group chains (a: chunks 0-1, b: chunks 2-3),
each owning a 2-bank PSUM tile.  With pool bufs=2, four half-chains are
in flight, which hides each PSUM->SBUF eviction's latency behind the
other chains' matmuls (the evictions, on ScalarE/VectorE at ~1 col/cyc
from PSUM, are the throughput wall of this dataflow: 3 layers x 2048
cols per group across two engines ~= 3.4 us/group).

  - layer 1 (K=6+1): the 4 chunks are packed into the four 32-row PE
    groups (tile_position row packing) and run concurrently; the bias is
    folded into the matmul via a constant ones-row in x (K=7).
  - layers 2/3 (K=128): one matmul per chunk, float32r (1 cycle/row).
  - layer 4 is computed transposed (h3-slice stationary, W4.T moving,
    N=4) with h3/W4 in bf16 so the 16 LDWEIGHTS per group get Fast
    Weight Load (2 cols/cycle); the group's output is a dense [128, 32]
    PSUM block per half, so its eviction is nearly free.  b4 is added on
    the host.
  - evictions are fused bias+ReLU ops; the a/b chain <-> ScalarE/VectorE
    assignment alternates by group parity.
  - all weights/biases ship as one packed [128, 388] input (W4's bf16
    pairs bitcast into two f32 columns) to minimize per-dispatch PJRT
    operand overhead.
"""

import numpy as np

N = 1048576
CORES = 8
R = N // CORES            # rows per core
CHUNK = 512               # rows per matmul (one PSUM bank of fp32)
GPC = 4                   # chunks per group
GROUPS = R // (CHUNK * GPC)   # 64
GW = GPC * CHUNK          # 2048 columns per group
REPEAT = 1                # times to run the whole compute body (bench only)

_CACHE = {}


def _build():
    import concourse.bacc as bacc
    import concourse.mybir as mybir
    import concourse.tile as tile

    f32 = mybir.dt.float32
    f32r = mybir.dt.float32r
    bf16 = mybir.dt.bfloat16
    Relu = mybir.ActivationFunctionType.Relu
    op_add = mybir.AluOpType.add
    op_max = mybir.AluOpType.max

    nc = bacc.Bacc("TRN2", target_bir_lowering=False, debug=False)

    xin = nc.dram_tensor(
        "xin", [GROUPS // 4, GPC, 7, 4 * CHUNK], f32r, kind="ExternalInput"
    ).ap()
    wpack = nc.dram_tensor(
        "wpack", [128, 390], f32r, kind="ExternalInput"
    ).ap()  # w1 | w2 | w3 | b2 | b3 | w4
    yout = nc.dram_tensor(
        "yout", [GROUPS // 4, 2, 128, 128], bf16, kind="ExternalOutput"
    ).ap()

    with tile.TileContext(nc) as tc:
        with (
            tc.tile_pool(name="const", bufs=1) as cpool,
            tc.tile_pool(name="x", bufs=4) as xpool,
            tc.tile_pool(name="h", bufs=6) as hpool,
            tc.tile_pool(name="o", bufs=4) as opool,
            tc.tile_pool(name="psum", bufs=2, space="PSUM") as ppool,
        ):
            wps = cpool.tile([128, 390], f32r, tag="wp")
            nc.sync.dma_start(out=wps[:], in_=wpack)
            # w4 ships as f32 inside wpack; cast to bf16 on device (SWDGE
            # cast DMA) so layer 4's LDWEIGHTS get Fast Weight Load
            w4t_ = cpool.tile([128, 4], bf16, tag="w4")
            nc.gpsimd.dma_start(
                out=w4t_[:, :], in_=wpack[:, 386:390].bitcast(f32)
            )
            w4s = w4t_[:, :]
            w1s = wps[:, 0:128]
            w2s = wps[:, 128:256]
            w3s = wps[:, 256:384]
            b2s = wps[:, 384:385].bitcast(f32)
            b3s = wps[:, 385:386].bitcast(f32)

            w1r = w1s.rearrange("(a b) c -> a b c", b=32)

            HW = GW // 2  # 1024 columns: each half-group (2 chunks)
            st = {}       # per-group in-flight tiles
            xts = {}      # x tile per 4-group block
            oab = {}      # output accumulation tiles per 4-group block

            def evict_relu(use_act, out_ap, in_ap, bias_ap):
                """bias+ReLU PSUM->SBUF eviction on either engine."""
                if use_act:
                    if bias_ap is None:
                        nc.scalar.activation(out_ap, in_ap, Relu)
                    else:
                        nc.scalar.activation(out_ap, in_ap, Relu, bias=bias_ap)
                elif bias_ap is None:
                    nc.vector.tensor_scalar(
                        out=out_ap,
                        in0=in_ap,
                        scalar1=0.0,
                        scalar2=None,
                        op0=op_max,
                    )
                else:
                    nc.vector.tensor_scalar(
                        out=out_ap,
                        in0=in_ap,
                        scalar1=bias_ap,
                        scalar2=0.0,
                        op0=op_add,
                        op1=op_max,
                    )

            def evict_copy(use_act, out_ap, in_ap):
                if use_act:
                    nc.scalar.activation(
                        out_ap, in_ap, mybir.ActivationFunctionType.Copy
                    )
                else:
                    nc.vector.tensor_copy(out=out_ap, in_=in_ap)

            def load_x(blk):
                """DMA one 4-group block of x into SBUF."""
                if blk >= GROUPS // 4 or blk in xts:
                    return
                xt = xpool.tile([128, 4 * CHUNK], f32r, tag="x")
                xtr = xt.rearrange("(a b) c -> a b c", b=32)
                for c in range(GPC):
                    nc.sync.dma_start(out=xtr[c, 0:7, :], in_=xin[blk, c])
                xts[blk] = xtr

            def front_a(g):
                """layer-1 matmuls + L1 eviction (x prefetched a block
                ahead so L1 never waits on the DMA)."""
                q = g % 4
                if q == 0:
                    load_x(g // 4)      # no-op except for block 0
                    load_x(g // 4 + 1)  # prefetch next block
                xtr = xts[g // 4]
                # two independent half-group chains: a (chunks 0-1, all
                # evictions on ScalarE) and b (chunks 2-3, VectorE).  Each
                # half owns a 2-bank PSUM tile; with bufs=2 four half-
                # chains are in flight, hiding each eviction's latency.
                pa = ppool.tile([128, HW], f32, tag="pa")
                pb = ppool.tile([128, HW], f32, tag="pb")
                for c in range(GPC):
                    dst = pa if c < 2 else pb
                    off = (c % 2) * CHUNK
                    nc.tensor.matmul(
                        dst[:, off : off + CHUNK],
                        lhsT=w1r[c, 0:7, :],
                        rhs=xtr[c, 0:7, q * CHUNK : (q + 1) * CHUNK],
                        start=True,
                        stop=True,
                        tile_position=(32 * c, 0),
                    )
                ha = hpool.tile([128, HW], f32r, tag="ha")
                hb = hpool.tile([128, HW], f32r, tag="hb")
                act_a = g % 2 == 0  # alternate engine<->chain to balance
                evict_relu(act_a, ha[:, :], pa[:, :], None)
                evict_relu(not act_a, hb[:, :], pb[:, :], None)
                st[g] = {"pa": pa, "pb": pb, "ha": ha, "hb": hb}

            def front_b(g):
                """layer-2 matmuls + L2 eviction."""
                s = st[g]
                for c in range(GPC):
                    dst = s["pa"] if c < 2 else s["pb"]
                    src_h = s["ha"] if c < 2 else s["hb"]
                    off = (c % 2) * CHUNK
                    nc.tensor.matmul(
                        dst[:, off : off + CHUNK],
                        lhsT=w2s,
                        rhs=src_h[:, off : off + CHUNK],
                        start=True,
                        stop=True,
                    )
                h2a = hpool.tile([128, HW], f32r, tag="h2a")
                h2b = hpool.tile([128, HW], f32r, tag="h2b")
                act_a = g % 2 == 0
                evict_relu(act_a, h2a[:, :], s["pa"][:, :], b2s)
                evict_relu(not act_a, h2b[:, :], s["pb"][:, :], b2s)
                s["h2a"], s["h2b"] = h2a, h2b

            def back_a(g):
                """layer-3 matmuls + L3 eviction (bf16 h3 -> FWL in L4)."""
                s = st[g]
                for c in range(GPC):
                    dst = s["pa"] if c < 2 else s["pb"]
                    src_h = s["h2a"] if c < 2 else s["h2b"]
                    off = (c % 2) * CHUNK
                    nc.tensor.matmul(
                        dst[:, off : off + CHUNK],
                        lhsT=w3s,
                        rhs=src_h[:, off : off + CHUNK],
                        start=True,
                        stop=True,
                    )
                h3a = hpool.tile([128, HW], bf16, tag="h3a")
                h3b = hpool.tile([128, HW], bf16, tag="h3b")
                act_a = g % 2 == 0
                evict_relu(act_a, h3a[:, :], s["pa"][:, :], b3s)
                evict_relu(not act_a, h3b[:, :], s["pb"][:, :], b3s)
                s["h3a"], s["h3b"] = h3a, h3b

            def back_b(g):
                """layer 4 (transposed, bf16 FWL), output copy + DMA."""
                s = st.pop(g)
                q = g % 4
                for sl in range(16):
                    dst = s["pa"] if sl < 8 else s["pb"]
                    src_h = s["h3a"] if sl < 8 else s["h3b"]
                    off = 128 * (sl % 8)
                    nc.tensor.matmul(
                        dst[:, 4 * (sl % 8) : 4 * (sl % 8) + 4],
                        lhsT=src_h[:, off : off + 128],
                        rhs=w4s[:, :],
                        start=True,
                        stop=True,
                        skip_group_check=True,
                    )
                if q == 0:
                    ota = opool.tile([128, 128], bf16, tag="oa")
                    otb = opool.tile([128, 128], bf16, tag="ob")
                    oab[g // 4] = (ota, otb)
                ota, otb = oab[g // 4]
                act_a = g % 2 == 0
                evict_copy(act_a, ota[:, 32 * q : 32 * q + 32], s["pa"][:, 0:32])
                evict_copy(not act_a, otb[:, 32 * q : 32 * q + 32], s["pb"][:, 0:32])
                if q == 3:
                    nc.sync.dma_start(out=yout[g // 4, 0], in_=ota[:])
                    nc.sync.dma_start(out=yout[g // 4, 1], in_=otb[:])
                    del oab[g // 4], xts[g // 4]

            # two-stage software pipeline over groups (the scheduler also
            # reorders by readiness; the pa/pb half-chains are what give
            # it slack to fill eviction waits)
            for gg in [g for _ in range(REPEAT) for g in range(GROUPS + 1)]:
                if gg < GROUPS:
                    front_a(gg)
                if gg >= 1:
                    back_a(gg - 1)
                if gg < GROUPS:
                    front_b(gg)
                if gg >= 1:
                    back_b(gg - 1)

    nc.compile()
    return nc


def _prep_in_maps(x, W1, b1, W2, b2, W3, b3, W4, b4):
    x = np.ascontiguousarray(np.asarray(x, dtype=np.float32))

    wp = np.zeros((128, 390), np.float32)
    W1T = np.asarray(W1, np.float32).T  # [6, 128]
    for g in range(GPC):
        wp[32 * g : 32 * g + 6, 0:128] = W1T
        wp[32 * g + 6, 0:128] = np.asarray(b1, np.float32)
    wp[:, 128:256] = np.asarray(W2, np.float32).T
    wp[:, 256:384] = np.asarray(W3, np.float32).T
    wp[:, 384] = np.asarray(b2, np.float32)
    wp[:, 385] = np.asarray(b3, np.float32)
    wp[:, 386:390] = np.asarray(W4, np.float32).T

    in_maps = []
    for c in range(CORES):
        xc = x[c * R : (c + 1) * R]  # [R, 6]
        # xin[xg, g, k, q*CHUNK + j] = xc[((xg*4 + q)*GPC + g)*CHUNK + j, k]
        xr = xc.reshape(GROUPS // 4, 4, GPC, CHUNK, 6).transpose(0, 2, 4, 1, 3)
        xr = xr.reshape(GROUPS // 4, GPC, 6, 4 * CHUNK)
        xi = np.empty((GROUPS // 4, GPC, 7, 4 * CHUNK), np.float32)
        xi[:, :, 0:6, :] = xr
        xi[:, :, 6, :] = 1.0
        in_maps.append({"xin": xi, "wpack": wp})
    return in_maps


def _execute(in_maps, trace=False):
    from concourse.bass_utils import run_bass_kernel_spmd

    if "nc" not in _CACHE:
        _CACHE["nc"] = _build()
    return run_bass_kernel_spmd(
        _CACHE["nc"], in_maps, list(range(CORES)), trace=trace
    )


def bench(in_maps, iters=20):
    """Measure the per-iteration device-side execution time of the kernel.

    The NeuronCores are reached through an axon tunnel whose host<->device
    round-trip latency is ~60 ms — three orders of magnitude above the
    kernel itself — so timing one synchronous dispatch measures the
    network, not the hardware.  Instead we enqueue N dispatches
    back-to-back (device-resident inputs, one final block_until_ready) so
    consecutive NEFF executions pipeline on-device, and recover the
    marginal per-iteration cost as the slope between a short and a long
    pipelined batch: slope = (T(N2) - T(N1)) / (N2 - N1).  The one-time
    tunnel round trip cancels in the difference.  Batches are repeated
    interleaved and min-aggregated to reject one-sided scheduling noise.

    Returns [slope_seconds] (list, for min() compatibility).
    """
    import time

    import jax
    from jax.experimental.shard_map import shard_map
    from jax.sharding import Mesh, NamedSharding, PartitionSpec

    import concourse.mybir as mybir
    from concourse import bass2jax

    if "nc" not in _CACHE:
        _CACHE["nc"] = _build()
    nc = _CACHE["nc"]
    bass2jax.install_neuronx_cc_hook()

    in_names, out_names, out_avals = [], [], []
    for alloc in nc.m.functions[0].allocations:
        if not isinstance(alloc, mybir.MemoryLocationSet):
            continue
        name = alloc.memorylocations[0].name
        pid = nc.partition_id_tensor.name if nc.partition_id_tensor else None
        if alloc.kind == "ExternalInput":
            if name != pid:
                in_names.append(name)
        elif alloc.kind == "ExternalOutput":
            out_names.append(name)
            out_avals.append(
                jax.core.ShapedArray(
                    tuple(alloc.tensor_shape), mybir.dt.np(alloc.dtype)
                )
            )
    n_params = len(in_names)
    all_names = tuple(in_names + out_names)

    def _body(*args):
        operands = list(args)
        if nc.partition_id_tensor is not None:
            operands.append(bass2jax.partition_id_tensor())
        outs = bass2jax._bass_exec_p.bind(
            *operands,
            out_avals=tuple(out_avals),
            in_names=all_names
            + ((nc.partition_id_tensor.name,) if nc.partition_id_tensor else ()),
            out_names=tuple(out_names),
            lowering_input_output_aliases=(),
            sim_require_finite=True,
            sim_require_nnan=True,
            nc=nc,
        )
        return tuple(outs)

    devices = jax.devices()[:CORES]
    mesh = Mesh(np.asarray(devices), ("core",))
    in_specs = (PartitionSpec("core"),) * (n_params + len(out_names))
    out_specs = (PartitionSpec("core"),) * len(out_names)
    sm = shard_map(
        _body, mesh=mesh, in_specs=in_specs, out_specs=out_specs, check_rep=False
    )

    concat_in = [
        np.concatenate([np.asarray(in_maps[c][n]) for c in range(CORES)], axis=0)
        for n in in_names
    ]
    zeros = [
        np.zeros((CORES * av.shape[0], *av.shape[1:]), av.dtype) for av in out_avals
    ]
    sh = NamedSharding(mesh, PartitionSpec("core"))
    dev_in = [jax.device_put(a, sh) for a in concat_in]
    dev_zeros = [jax.device_put(z, sh) for z in zeros]

    fn = bass2jax.fast_dispatch_compile(
        lambda: jax.jit(sm, keep_unused=True).lower(*dev_in, *dev_zeros).compile()
    )

    def batch(n):
        t0 = time.perf_counter()
        out = None
        for _ in range(n):
            out = fn(*dev_in, *dev_zeros)
        jax.block_until_ready(out)
        return time.perf_counter() - t0

    batch(2)  # warmup
    n1, n2, reps = 10, 100, max(8, iters // 3)
    t1s, t2s = [], []
    for _ in range(reps):
        t1s.append(batch(n1))
        t2s.append(batch(n2))
    slope = (min(t2s) - min(t1s)) / (n2 - n1)
    slope = max(slope, 1e-9)
    print(
        f"bench: T({n1}) {[round(t * 1e3, 2) for t in t1s]} ms, "
        f"T({n2}) {[round(t * 1e3, 2) for t in t2s]} ms"
    )
    return [slope]


def kernel(**inputs):
    in_maps = _prep_in_maps(
        inputs["x"],
        inputs["W1"],
        inputs["b1"],
        inputs["W2"],
        inputs["b2"],
        inputs["W3"],
        inputs["b3"],
        inputs["W4"],
        inputs["b4"],
    )
    results = _execute(in_maps).results
    outs = []
    for c in range(CORES):
        # yout dims: (xg, half, p, (q, s4, k)); group = xg*4 + q,
        # row = group*2048 + half*1024 + s4*128 + p
        yo = (
            np.asarray(results[c]["yout"])
            .astype(np.float32)
            .reshape(GROUPS // 4, 2, 128, 4, 8, 4)
        )
        outs.append(yo.transpose(0, 3, 1, 4, 2, 5).reshape(R, 4))
    y = np.concatenate(outs, axis=0)
    y += np.asarray(inputs["b4"], np.float32)  # layer-4 bias, added on host
    return np.ascontiguousarray(y.astype(np.float32))

